# revision 20
# baseline (speedup 1.0000x reference)
"""Bass/Trainium2 kernel for nn_AttentionLayer_68229850464552.

Full multi-head causal attention layer (QKV proj + partial RoPE + attention +
output proj), head-sharded (tensor parallel) across 8 NeuronCores. Each core
computes 2 of the 16 heads for both batch elements and the partial output
projection for its heads' feature columns; the host sums the 8 partials and
adds the output bias.

Matmul operands are bf16 (PE streams 2B/lane/cycle -> 1 cycle/row; fp32/fp32r
stream at half rate); accumulation is fp32 in PSUM throughout.

Self-contained: hardcodes shapes from the problem spec.
"""
import os
import numpy as np
import ml_dtypes
from contextlib import ExitStack

import concourse.bass as bass
import concourse.mybir as mybir
import concourse.tile as tile
from concourse import bacc
from concourse.bass_utils import run_bass_kernel_spmd

B, S, D, H, DK = 2, 2048, 2048, 16, 128
HPC = 2                      # heads per core
NCORES = 8
DR = 32                      # rope features
SCALE = 1.0 / float(np.sqrt(DK))
CH = 512                     # x seq-chunk width for the QKV projection
NCH = S // CH                # 4
QCW = 512                    # query chunk width in attention
NQC = S // QCW               # 4
NJ = S // 128                # 16 key blocks
WQ_COLS = 4 * 128            # q0,q1,k0,k1 M-tiles
WV_COLS = 2 * 129            # [v_h0 | ones] [v_h1 | ones]

F32 = mybir.dt.float32
BF16 = mybir.dt.bfloat16
Act = mybir.ActivationFunctionType
Alu = mybir.AluOpType
BF_NP = ml_dtypes.bfloat16

_PROG_CACHE = {}


def _build_program():
    nc = bacc.Bacc("TRN2", target_bir_lowering=False, debug=False,
                   enable_asserts=True, num_devices=NCORES)

    xT = nc.dram_tensor("xT", [B, D, S], BF16, kind="ExternalInput").ap()
    wq = nc.dram_tensor("wq", [D, WQ_COLS], BF16, kind="ExternalInput").ap()
    wv = nc.dram_tensor("wv", [D, WV_COLS], BF16, kind="ExternalInput").ap()
    wo = nc.dram_tensor("wo", [HPC * DK, D], BF16, kind="ExternalInput").ap()
    bqk = nc.dram_tensor("bqk", [128, 4], F32, kind="ExternalInput").ap()
    bv = nc.dram_tensor("bv", [128, WV_COLS], F32, kind="ExternalInput").ap()
    cosT = nc.dram_tensor("cosT", [DR, S], F32, kind="ExternalInput").ap()
    sinT = nc.dram_tensor("sinT", [DR, S], F32, kind="ExternalInput").ap()
    maskT = nc.dram_tensor("maskT", [128, 128], BF16, kind="ExternalInput").ap()
    idm = nc.dram_tensor("idm", [128, 128], BF16, kind="ExternalInput").ap()
    pout = nc.dram_tensor("pout", [B * S, D], F32, kind="ExternalOutput").ap()
    scratch = nc.dram_tensor("scratch", [64, 64], F32, kind="ExternalOutput").ap()

    with tile.TileContext(nc) as tc, ExitStack() as ctx:
        wpool = ctx.enter_context(tc.tile_pool(name="w", bufs=1))
        xpool = ctx.enter_context(tc.tile_pool(name="x", bufs=3))
        qkpool = ctx.enter_context(tc.tile_pool(name="qk", bufs=1))
        vpool = ctx.enter_context(tc.tile_pool(name="v", bufs=1))
        otpool = ctx.enter_context(tc.tile_pool(name="ot", bufs=1))
        ppool = ctx.enter_context(tc.tile_pool(name="p", bufs=4))
        rpool = ctx.enter_context(tc.tile_pool(name="r", bufs=3))
        opool = ctx.enter_context(tc.tile_pool(name="o", bufs=3))
        scpool = ctx.enter_context(tc.tile_pool(name="sc", bufs=2, space="PSUM"))
        accpool = ctx.enter_context(tc.tile_pool(name="acc", bufs=3, space="PSUM"))
        pjpool = ctx.enter_context(tc.tile_pool(name="pj", bufs=3, space="PSUM"))

        # resident weights / constants
        wq_sb = wpool.tile([128, 16, WQ_COLS], BF16)
        nc.sync.dma_start(wq_sb[:, :, 0:128],
                          wq[:, 0:128].rearrange("(kt p) m -> p kt m", p=128))
        xt0 = xpool.tile([128, 16, CH], BF16, tag="xt", name="xt0")
        nc.sync.dma_start(xt0[:], xT[0, :, 0:CH].rearrange(
            "(kt p) s -> p kt s", p=128))
        bqk_sb = wpool.tile([128, 4], F32)
        nc.sync.dma_start(bqk_sb[:], bqk[:])
        nc.sync.dma_start(wq_sb[:, :, 128:WQ_COLS],
                          wq[:, 128:WQ_COLS].rearrange(
                              "(kt p) m -> p kt m", p=128))
        wv_sb = wpool.tile([128, 16, WV_COLS], BF16)
        nc.sync.dma_start(wv_sb[:], wv.rearrange("(kt p) m -> p kt m", p=128))
        bv_sb = wpool.tile([128, WV_COLS], F32)
        nc.sync.dma_start(bv_sb[:], bv[:])
        cos_sb = wpool.tile([DR, S], F32)
        nc.sync.dma_start(cos_sb[:], cosT[:])
        sin_sb = wpool.tile([DR, S], F32)
        nc.sync.dma_start(sin_sb[:], sinT[:])
        maskT_sb = wpool.tile([128, 128], BF16)
        nc.sync.dma_start(maskT_sb[:], maskT[:])
        idm_sb = wpool.tile([128, 128], BF16)
        nc.sync.dma_start(idm_sb[:], idm[:])
        ones_sb = wpool.tile([128, 128], BF16)
        nc.gpsimd.memset(ones_sb[:], 1.0)

        # PE warmup: keep the tensor engine busy while input DMAs land so the
        # HAM clock gate reaches 8/8 before real matmuls start
        wps = pjpool.tile([64, 64], F32, tag="pj", name="warm")
        for w in range(220):
            nc.tensor.matmul(wps[:], ones_sb[0:128, 0:64], ones_sb[0:128, 0:64],
                             start=(w == 0), stop=(w == 219))
        wsb = rpool.tile([64, 64], F32, tag="warmsb", name="warmsb")
        nc.vector.tensor_copy(wsb[:], wps[:])
        nc.sync.dma_start(scratch[:], wsb[:])
        wo_sb = wpool.tile([128, 2, D], BF16)
        nc.sync.dma_start(wo_sb[:], wo.rearrange("(kt p) m -> p kt m", p=128))

        def outproj_fn(b, ot_sb):
            def _outproj(qc=NQC - 1):
                for sblk in range(4 * qc, 4 * qc + 4):
                    for n in range(D // 512):
                        ps = pjpool.tile([128, 512], F32, tag="pj",
                                         name="psC")
                        for kt in range(2):
                            nc.tensor.matmul(
                                ps[:],
                                ot_sb[:, kt, sblk * 128:(sblk + 1) * 128],
                                wo_sb[:, kt, n * 512:(n + 1) * 512],
                                start=(kt == 0), stop=(kt == 1))
                        po = opool.tile([128, 512], F32, tag="po", name="po")
                        if (sblk + n) % 2 == 0:
                            nc.vector.tensor_copy(po[:], ps[:])
                        else:
                            nc.scalar.activation(po[:], ps[:], Act.Copy)
                        nc.sync.dma_start(
                            pout[b * S + sblk * 128: b * S + (sblk + 1) * 128,
                                 n * 512:(n + 1) * 512], po[:])
            return _outproj

        pending = []
        for b in range(B):
            # ---------------- Phase A: QKV projection + RoPE ----------------
            # qk_sb[t]: [feat(128), S] for t in (q_h0, q_h1, k_h0, k_h1)
            qk_sb = [qkpool.tile([128, S], BF16, tag=f"qk{t}", name=f"qk{t}")
                     for t in range(4)]
            v_sb = vpool.tile([128, NJ, WV_COLS], BF16, tag="v")

            for c in range(NCH):
                cs = slice(c * CH, (c + 1) * CH)
                if b == 0 and c == 0:
                    xt = xt0
                else:
                    xt = xpool.tile([128, 16, CH], BF16, tag="xt")
                    nc.sync.dma_start(
                        xt[:], xT[b, :, cs].rearrange("(kt p) s -> p kt s", p=128))

                for mt in range(4):
                    ps = pjpool.tile([128, CH], F32, tag="pj")
                    for kt in range(16):
                        nc.tensor.matmul(
                            ps[:], wq_sb[:, kt, mt * 128:(mt + 1) * 128],
                            xt[:, kt, :], start=(kt == 0), stop=(kt == 15))
                    nc.scalar.activation(qk_sb[mt][:, cs], ps[:],
                                         Act.Identity,
                                         bias=bqk_sb[:, mt:mt + 1])

                # RoPE on the first DR features of each q/k tensor, per chunk:
                # rot = [q[16:32] (sign folded into sinT), q[0:16]]
                for t4 in range(4):
                    shuf = rpool.tile([DR, CH], BF16, tag="shuf", name="shuf")
                    nc.sync.dma_start(shuf[0:16, :], qk_sb[t4][16:32, cs])
                    nc.sync.dma_start(shuf[16:32, :], qk_sb[t4][0:16, cs])
                    tmp = rpool.tile([DR, CH], F32, tag="rt", name="tmp")
                    nc.vector.tensor_tensor(tmp[:], shuf[:], sin_sb[:, cs],
                                            Alu.mult)
                    tgt = qk_sb[t4][0:DR, cs]
                    nc.vector.tensor_tensor(tgt, tgt, cos_sb[:, cs], Alu.mult)
                    nc.vector.tensor_tensor(tgt, tgt, tmp[:], Alu.add)

                # V projection for this chunk ([seq, feat] layout, + ones col)
                for s2 in range(CH // 128):
                    psv = pjpool.tile([128, WV_COLS], F32, tag="pj")
                    for kt in range(16):
                        nc.tensor.matmul(
                            psv[:], xt[:, kt, s2 * 128:(s2 + 1) * 128],
                            wv_sb[:, kt, :], start=(kt == 0), stop=(kt == 15))
                    nc.vector.tensor_tensor(
                        v_sb[:, c * (CH // 128) + s2, :], psv[:],
                        bv_sb[:], Alu.add)

                if c == 0 and pending:
                    pending.pop(0)()

            # ------- Phase B + C: attention, pipelined with out-proj --------
            ot_sb = otpool.tile([128, HPC, S], BF16, tag="ot")
            ot_raw = otpool.tile([128, NQC * HPC, QCW], F32, tag="otr")
            sums_all = otpool.tile([128, NQC * HPC, QCW], F32, tag="sma")

            def norm_h(qc, h):
                i_qh = qc * HPC + h
                nc.vector.reciprocal(sums_all[:, i_qh, :],
                                     sums_all[:, i_qh, :])
                nc.gpsimd.tensor_tensor(
                    ot_sb[:, h, qc * QCW:(qc + 1) * QCW],
                    ot_raw[:, i_qh, :], sums_all[:, i_qh, :], Alu.mult)

            def outproj(qc):
                outproj_fn(b, ot_sb)(qc)

            prev_unit = None
            for qc in range(NQC):
                jmax = 4 * qc + 3
                for h in range(HPC):
                    i_qh = qc * HPC + h
                    otps = accpool.tile([128, QCW], F32, tag="acc")
                    sums = accpool.tile([128, QCW], F32, tag="acc")

                    def emit_score(j):
                        c0 = (j - 4 * qc) * 128 if j >= 4 * qc else 0
                        diag = j >= 4 * qc
                        sps = scpool.tile([128, QCW], F32, tag="sc",
                                          name="sps")
                        nc.tensor.matmul(
                            sps[:, c0:QCW], qk_sb[2 + h][:, j * 128:(j + 1) * 128],
                            qk_sb[h][:, qc * QCW + c0:(qc + 1) * QCW],
                            start=True, stop=not diag)
                        if diag:
                            # add -1e4 above the diagonal of the diag subblock
                            nc.tensor.matmul(
                                sps[:, c0:c0 + 128], maskT_sb[:], idm_sb[:],
                                start=False, stop=True)
                        return sps

                    def emit_consume(j, sps):
                        c0 = (j - 4 * qc) * 128 if j >= 4 * qc else 0
                        pt = ppool.tile([128, QCW], BF16, tag="pt", name="pt")
                        nc.scalar.activation(pt[:, c0:QCW], sps[:, c0:QCW],
                                             Act.Exp, scale=SCALE)
                        nc.tensor.matmul(
                            otps[:, c0:QCW],
                            v_sb[:, j, 129 * h:129 * h + 128],
                            pt[:, c0:QCW], start=(j == 0), stop=(j == jmax))
                        nc.tensor.matmul(
                            sums[:, c0:QCW], ones_sb[:],
                            pt[:, c0:QCW], start=(j == 0), stop=(j == jmax))

                    prev = emit_score(0)
                    for j in range(1, jmax + 1):
                        cur = emit_score(j)
                        emit_consume(j - 1, prev)
                        prev = cur
                    emit_consume(jmax, prev)
                    nc.scalar.activation(sums_all[:, i_qh, :], sums[:], Act.Copy)
                    nc.vector.tensor_copy(ot_raw[:, i_qh, :], otps[:])
                    if prev_unit is not None:
                        norm_h(*prev_unit)
                    prev_unit = (qc, h)
                if qc >= 1:
                    outproj(qc - 1)
            norm_h(*prev_unit)
            pending.append(outproj_fn(b, ot_sb))
        while pending:
            pending.pop(0)()

    nc.compile()
    return nc


def kernel(x, W_qkv, b_qkv, W_out, b_out):
    x = np.asarray(x, dtype=np.float32)
    W_qkv = np.asarray(W_qkv, dtype=np.float32)
    b_qkv = np.asarray(b_qkv, dtype=np.float32)
    W_out = np.asarray(W_out, dtype=np.float32)
    b_out = np.asarray(b_out, dtype=np.float32)

    if "prog" not in _PROG_CACHE:
        _PROG_CACHE["prog"] = _build_program()
    nc = _PROG_CACHE["prog"]

    xT = np.ascontiguousarray(x.transpose(0, 2, 1)).astype(BF_NP)

    i = np.arange(16, dtype=np.float64)
    theta = 1.0 / (10000.0 ** ((2.0 * i) / DR))
    s_idx = np.arange(S, dtype=np.float64)
    idx = s_idx[:, None] * theta[None, :]          # [S, 16]
    idx2 = np.concatenate([idx, idx], axis=1)      # [S, 32]
    cosT = np.ascontiguousarray(np.cos(idx2).T.astype(np.float32))
    sinT = np.sin(idx2).T.astype(np.float32)
    sinT[0:16, :] *= -1.0          # sign of rot = [-q[16:32], q[0:16]] folded in
    sinT = np.ascontiguousarray(sinT)

    maskT = np.triu(np.full((128, 128), -10000.0, dtype=np.float32), 1).astype(BF_NP)
    idm = np.eye(128, dtype=np.float32).astype(BF_NP)

    in_maps = []
    for c in range(NCORES):
        heads = [HPC * c, HPC * c + 1]
        qw, kw, vw, qb, kb, vb = [], [], [], [], [], []
        for hh in heads:
            base = 3 * DK * hh
            qw.append(W_qkv[base:base + 128])
            kw.append(W_qkv[base + 128:base + 256])
            vw.append(W_qkv[base + 256:base + 384])
            qb.append(b_qkv[base:base + 128])
            kb.append(b_qkv[base + 128:base + 256])
            vb.append(b_qkv[base + 256:base + 384])

        wq_np = np.ascontiguousarray(
            np.concatenate([qw[0], qw[1], kw[0], kw[1]],
                           axis=0).T).astype(BF_NP)             # [D, 512]

        Mv = np.zeros((WV_COLS, D), dtype=np.float32)
        Mv[0:128] = vw[0]
        Mv[129:257] = vw[1]
        wv_np = np.ascontiguousarray(Mv.T).astype(BF_NP)        # [D, 258]

        bv_np = np.zeros((1, WV_COLS), dtype=np.float32)
        bv_np[0, 0:128] = vb[0]
        bv_np[0, 128] = 1.0
        bv_np[0, 129:257] = vb[1]
        bv_np[0, 257] = 1.0
        bv_np = np.ascontiguousarray(np.repeat(bv_np, 128, axis=0))

        bqk_np = np.zeros((128, 4), dtype=np.float32)
        bqk_np[:, 0] = qb[0]
        bqk_np[:, 1] = qb[1]
        bqk_np[:, 2] = kb[0]
        bqk_np[:, 3] = kb[1]

        wo_np = np.ascontiguousarray(
            W_out[:, HPC * DK * c: HPC * DK * (c + 1)].T).astype(BF_NP)

        in_maps.append({
            "xT": xT, "wq": wq_np, "wv": wv_np, "wo": wo_np,
            "bqk": bqk_np, "bv": bv_np, "cosT": cosT, "sinT": sinT,
            "maskT": maskT, "idm": idm,
        })

    trace = os.environ.get("KERNEL_TRACE", "0") == "1"
    res = run_bass_kernel_spmd(nc, in_maps, core_ids=list(range(NCORES)),
                               trace=trace)
    if res.exec_time_ns is not None:
        print(f"HW exec time: {res.exec_time_ns} ns")
        if res.instructions_and_trace is not None:
            print(f"trace: {res.instructions_and_trace[1]}")

    acc = np.zeros((B * S, D), dtype=np.float64)
    for c in range(NCORES):
        acc += res.results[c]["pout"].astype(np.float64)
    out = (acc + b_out.astype(np.float64)[None, :]).astype(np.float32)
    return out.reshape(B, S, D)
